# revision 1
# baseline (speedup 1.0000x reference)
"""Trainium2 Bass kernel for nn_AttentionBlock (Swin-style window attention,
16x16 windows, 16 heads, head_dim 32, cosine-distance post-softmax modulation).

Strategy: pure data-parallel over 8 NeuronCores (16 windows each). All
layouts are chosen so no on-chip transposes are needed:

  - x is pre-transposed on host to xT [C, tokens]  (bf16)
  - qkv is computed in [c_out, token] layout for q,k (so q,k land as [d, n]
    per head with 4 consecutive heads at partition offsets 0/32/64/96), and
    in [token, c_out] layout for v.
  - attention scores are computed *transposed*: S^T[m, n] = sum_d k[d,m] q[d,n]
    with the relative-position bias added by an identity-matmul PSUM
    accumulation (which also performs the bank's has_written clear).
  - softmax denominators are computed with an all-ones [128, 32] stationary
    matmul (replicating each head's denominator across 32 partitions, so the
    final normalization is a plain tensor_tensor multiply).
  - exp has no max-subtraction: logits are bounded (~|2|) for this problem.
  - post-softmax modulation MOD (and nothing else) is a single bf16
    tensor_tensor multiply per pair.
  - PV matmul: out^T[d, n] = sum_m v[m, d] P^T[m, n], col-tiled 4 heads into
    one PSUM tile; attention output is produced directly in [c, token] layout
    which feeds the final projection without transposes.
  - final y^T [c_out, token] is DMA'd out; host transposes back.
"""

import math
import sys

import numpy as np

for _p in ("/opt/trn_rl_repo",):
    if _p not in sys.path:
        sys.path.insert(0, _p)

import ml_dtypes  # noqa: E402

import concourse.bass as bass  # noqa: E402
import concourse.mybir as mybir  # noqa: E402
from concourse import bacc, tile  # noqa: E402
from concourse.bass_utils import run_bass_kernel_spmd  # noqa: E402
from concourse.masks import make_identity  # noqa: E402

BF16 = mybir.dt.bfloat16
F32 = mybir.dt.float32
NPBF16 = ml_dtypes.bfloat16

R = 16          # window side
N = R * R       # tokens per window = 256
H = 16          # heads
D = 32          # head dim
C = H * D       # 512
B_GLOB = 128    # windows total
NCORES = 8
B_LOC = B_GLOB // NCORES   # 16 windows per core
T_LOC = B_LOC * N          # 4096 tokens per core
SCALE = D ** -0.5

# Tunables (shared by _build and _prep_consts).
OPTS = {
    "exp_batch": 1,       # heads per exp/t3 op (1 or 2)
    "qk_copy": "dve",     # engine for qk PSUM->SBUF copyback: 'act' | 'dve'
    "y_copy": "dve",      # engine for y PSUM->SBUF copyback: 'act' | 'dve'
    "bias_mode": "pe",    # 'pe' (identity-matmul add) | 'dve' (exp-bias mul)
    "bias_dr": True,      # fp8 DoubleRow residual-pair bias matmul
    "p0_bufs": 6,
    "t3_bufs": 6,
}


def _rel_pos_index(r):
    coords = np.stack(np.meshgrid(np.arange(r), np.arange(r), indexing="ij"))
    cf = coords.reshape(2, -1)
    rel = cf[:, :, None] - cf[:, None, :]
    rel = rel.transpose(1, 2, 0).astype(np.int64)
    rel[:, :, 0] += r - 1
    rel[:, :, 1] += r - 1
    rel[:, :, 0] *= 2 * r - 1
    return rel.sum(-1)  # [N, N]


def _modulation(n, k):
    idx = np.arange(n * n)
    rr, cc = idx // n, idx % n
    d = np.sqrt((rr[:, None] - rr[None, :]) ** 2 + (cc[:, None] - cc[None, :]) ** 2)
    t = 4 * (n - 1) * math.sqrt(2)
    f = 2 * math.pi / t
    m = np.exp(np.cos(f * d)) / 2
    if k % n == 0:
        k = k - 1
    bound = m[0, k]
    m = np.where(m < bound, 0.0, m)
    return m.astype(np.float32)  # [N, N]


_REL_IDX = _rel_pos_index(R)
_MOD = _modulation(R, 3 * R)

_CACHE = {}


def _build():
    """Build the single-core Bass graph (SPMD: same NEFF on all 8 cores)."""
    o = OPTS
    eb = o["exp_batch"]
    assert eb in (1, 2)
    bias_pe = o["bias_mode"] == "pe"

    nc = bacc.Bacc(None, target_bir_lowering=False)

    xt = nc.declare_dram_parameter("xt", [128, 4, T_LOC], BF16, isOutput=False)
    wqk = nc.declare_dram_parameter("wqk", [128, 4, 1024], BF16, isOutput=False)
    wv = nc.declare_dram_parameter("wv", [128, 4, 512], BF16, isOutput=False)
    wp = nc.declare_dram_parameter("wp", [128, 4, 512], BF16, isOutput=False)
    FP8 = mybir.dt.float8e4
    bias_dr = o["bias_dr"] and bias_pe
    if bias_dr:
        biastp = nc.declare_dram_parameter("biastp", [128, H, 2, 512], FP8,
                                           isOutput=False)
        identdr = nc.declare_dram_parameter("identdr", [128, 2, 128], FP8,
                                            isOutput=False)
    else:
        biastp = nc.declare_dram_parameter("biastp", [128, H, 512], BF16,
                                           isOutput=False)
    modtp = nc.declare_dram_parameter("modtp", [128, 512], BF16, isOutput=False)
    bvb = nc.declare_dram_parameter("bvb", [128, 512], BF16, isOutput=False)
    qkb = nc.declare_dram_parameter("qkb", [128, 8], F32, isOutput=False)
    pb = nc.declare_dram_parameter("pb", [128, 4], F32, isOutput=False)
    out = nc.declare_dram_parameter("out", [4, 128, T_LOC], F32, isOutput=True)

    AF = mybir.ActivationFunctionType

    with tile.TileContext(nc) as tc:
        with (
            tc.tile_pool(name="const", bufs=1) as const,
            tc.tile_pool(name="qkp", bufs=2) as qkp,
            tc.tile_pool(name="vp", bufs=2) as vp,
            tc.tile_pool(name="p0p", bufs=o["p0_bufs"]) as p0p,
            tc.tile_pool(name="t3p", bufs=o["t3_bufs"]) as t3p,
            tc.tile_pool(name="rcp", bufs=2) as rcp,
            tc.tile_pool(name="aop", bufs=2) as aop,
            tc.tile_pool(name="yp", bufs=4) as yp,
            tc.tile_pool(name="ps_mm", bufs=2, space="PSUM") as ps_mm,
            tc.tile_pool(name="ps_s", bufs=(4 // eb), space="PSUM") as ps_s,
            tc.tile_pool(name="ps_o", bufs=2, space="PSUM") as ps_o,
        ):
            # ---- resident constants ----
            wqk_sb = const.tile([128, 4, 1024], BF16, name="wqk_sb")
            nc.sync.dma_start(out=wqk_sb[:], in_=wqk[:])
            wv_sb = const.tile([128, 4, 512], BF16, name="wv_sb")
            nc.sync.dma_start(out=wv_sb[:], in_=wv[:])
            wp_sb = const.tile([128, 4, 512], BF16, name="wp_sb")
            nc.sync.dma_start(out=wp_sb[:], in_=wp[:])
            if bias_dr:
                biastp_sb = const.tile([128, H, 2, 512], FP8, name="biastp_sb")
                nc.sync.dma_start(out=biastp_sb[:], in_=biastp[:])
                identdr_sb = const.tile([128, 2, 128], FP8, name="identdr_sb")
                nc.sync.dma_start(out=identdr_sb[:], in_=identdr[:])
            else:
                biastp_sb = const.tile([128, H, 512], BF16, name="biastp_sb")
                nc.sync.dma_start(out=biastp_sb[:], in_=biastp[:])
            modtp_sb = const.tile([128, eb * 512], BF16, name="modtp_sb")
            for _e in range(eb):
                nc.sync.dma_start(out=modtp_sb[:, _e * 512:(_e + 1) * 512],
                                  in_=modtp[:])
            bvb_sb = const.tile([128, 512], BF16, name="bvb_sb")
            nc.sync.dma_start(out=bvb_sb[:], in_=bvb[:])
            qkb_sb = const.tile([128, 8], F32, name="qkb_sb")
            nc.sync.dma_start(out=qkb_sb[:], in_=qkb[:])
            pb_sb = const.tile([128, 4], F32, name="pb_sb")
            nc.sync.dma_start(out=pb_sb[:], in_=pb[:])
            xt_sb = const.tile([128, 4, T_LOC], BF16, name="xt_sb")
            for _b in range(B_LOC):
                _c = slice(_b * N, (_b + 1) * N)
                nc.sync.dma_start(out=xt_sb[:, :, _c], in_=xt[:, :, _c])

            ident = const.tile([128, 128], BF16, name="ident")
            make_identity(nc, ident)
            ones32 = const.tile([128, 32], BF16, name="ones32")
            nc.gpsimd.memset(ones32, 1.0)
            zeros128 = const.tile([128, 128], BF16, name="zeros128")
            zerosdr = const.tile([128, 2, 256], mybir.dt.float8e4,
                                 name="zerosdr")
            nc.gpsimd.memset(zerosdr, 0.0)
            nc.gpsimd.memset(zeros128, 0.0)

            for b in range(B_LOC):
                col = slice(b * N, (b + 1) * N)
                e2 = b % 2
                # ---- qkv for a PAIR of windows: q,k in [c_out, t] layout,
                # N=512 matmuls (half the instructions / weight loads) ----
                if e2 == 0:
                    col2 = slice(b * N, (b + 2) * N)
                    qk_sb = qkp.tile([128, 8, 2 * N], BF16, name="qk_sb")
                    for cb in range(8):
                        ps = ps_mm.tile([128, 512], F32, name="ps_g",
                                        tag="ps_g")
                        for ci in range(4):
                            nc.tensor.matmul(
                                ps,
                                lhsT=wqk_sb[:, ci, cb * 128:(cb + 1) * 128],
                                rhs=xt_sb[:, ci, col2],
                                start=(ci == 0),
                                stop=(ci == 3),
                            )
                        if o["qk_copy"] == "act":
                            nc.scalar.activation(
                                qk_sb[:, cb, :], ps, AF.Identity,
                                bias=qkb_sb[:, cb:cb + 1],
                            )
                        else:
                            nc.vector.tensor_scalar_add(
                                qk_sb[:, cb, :], ps, qkb_sb[:, cb:cb + 1])
                # ---- v in [t, c_out] layout (bias via identity matmul) ----
                v_sb = vp.tile([128, 2, 512], BF16, name="v_sb")
                for tb in range(2):
                    ps = ps_mm.tile([128, 512], F32, name="ps_g", tag="ps_g")
                    nc.tensor.matmul(ps, lhsT=ident, rhs=bvb_sb,
                                     start=True, stop=False)
                    tcol = slice(b * N + tb * 128, b * N + (tb + 1) * 128)
                    for ci in range(4):
                        nc.tensor.matmul(
                            ps,
                            lhsT=xt_sb[:, ci, tcol],
                            rhs=wv_sb[:, ci, :],
                            start=False,
                            stop=(ci == 3),
                        )
                    nc.vector.tensor_copy(v_sb[:, tb, :], ps)

                # ---- attention, 4 heads per group (col/row-packed) ----
                ao_sb = aop.tile([128, 4, N], BF16, name="ao_sb")
                for g in range(4):
                    p0s = {}
                    t3s = {}
                    for jj in range(4 // eb):
                        ss = ps_s.tile([128, eb * 512], F32, name="ps_sT")
                        for e in range(eb):
                            j = jj * eb + e
                            h = 4 * g + j
                            hs = slice(e * 512, (e + 1) * 512)
                            if bias_dr:
                                # fp8 DoubleRow residual pair: adds
                                # biasA + biasB in one half-rate matmul
                                nc.tensor.matmul(
                                    ss[:, hs], lhsT=identdr_sb[:],
                                    rhs=biastp_sb[:, h],
                                    start=True, stop=False,
                                    perf_mode=mybir.MatmulPerfMode.DoubleRow)
                            elif bias_pe:
                                # bias add + full-bank has_written clear
                                nc.tensor.matmul(ss[:, hs], lhsT=ident,
                                                 rhs=biastp_sb[:, h, :],
                                                 start=True, stop=False)
                            else:
                                nc.tensor.matmul(ss[:, hs], lhsT=zeros128,
                                                 rhs=bvb_sb,
                                                 start=True, stop=False,
                                                 skip_group_check=True)
                            for c in range(2):
                                nc.tensor.matmul(
                                    ss[:, e * 512 + c * N:e * 512 + (c + 1) * N],
                                    lhsT=qk_sb[32 * j:32 * (j + 1), 4 + g,
                                               e2 * N + c * 128:
                                               e2 * N + (c + 1) * 128],
                                    rhs=qk_sb[32 * j:32 * (j + 1), g,
                                              e2 * N:(e2 + 1) * N],
                                    start=False,
                                    stop=(c == 1),
                                    tile_position=(32 * j, 0),
                                    skip_group_check=not bias_pe,
                                )
                        p0 = p0p.tile([128, eb * 512], BF16, name="p0")
                        nc.scalar.activation(p0, ss, AF.Exp)
                        if not bias_pe:
                            # multiply exp(S^T) by exp(bias^T) on DVE
                            t2 = t3p.tile([128, eb * 512], BF16, name="t3",
                                          tag="t3")
                            for e in range(eb):
                                h = 4 * g + jj * eb + e
                                nc.vector.tensor_mul(
                                    t2[:, e * 512:(e + 1) * 512],
                                    in0=p0[:, e * 512:(e + 1) * 512],
                                    in1=biastp_sb[:, h, :])
                            p0 = t2
                        t3 = t3p.tile([128, eb * 512], BF16, name="t3",
                                      tag="t3")
                        nc.vector.tensor_mul(t3, in0=p0, in1=modtp_sb[:, :eb * 512])
                        for e in range(eb):
                            j = jj * eb + e
                            p0s[j] = p0[:, e * 512:(e + 1) * 512]
                            t3s[j] = t3[:, e * 512:(e + 1) * 512]

                    po = ps_o.tile([128, 512], F32, name="ps_out")
                    # Half-bank zero-fill over the PV region: start=True marks
                    # the whole bank's has_written clear and zeroes cols 0:N.
                    # PV matmuls overlap it (auto-ordered); the ones-matmuls
                    # (cols N:2N, overwrite-on-pending) get explicit deps.
                    if bias_dr:
                        opener = nc.tensor.matmul(
                            po[:, 0:N], lhsT=identdr_sb[:], rhs=zerosdr[:],
                            start=True, stop=False, skip_group_check=True,
                            perf_mode=mybir.MatmulPerfMode.DoubleRow)
                    else:
                        opener = nc.tensor.matmul(po[:, 0:N], lhsT=zeros128,
                                                  rhs=bvb_sb[:, 0:N],
                                                  start=True, stop=False,
                                                  skip_group_check=True)
                    from concourse.tile_rust import add_dep_helper
                    for j in range(4):
                        h = 4 * g + j
                        orow = slice(32 * j, 32 * (j + 1))
                        for c in range(2):
                            nc.tensor.matmul(
                                po[orow, 0:N],
                                lhsT=v_sb[:, c, 32 * h:32 * (h + 1)],
                                rhs=t3s[j][:, c * N:(c + 1) * N],
                                start=False,
                                stop=(c == 1),
                                tile_position=(0, 32 * j),
                                skip_group_check=True,
                            )
                        for c in range(2):
                            mm = nc.tensor.matmul(
                                po[orow, N:2 * N],
                                lhsT=ones32,
                                rhs=p0s[j][:, c * N:(c + 1) * N],
                                start=False,
                                stop=(c == 1),
                                tile_position=(0, 32 * j),
                                skip_group_check=True,
                            )
                            if c == 0:
                                add_dep_helper(mm.ins, opener.ins, sync=False,
                                               reason="psum bank opener order")
                    recip = rcp.tile([128, N], F32, name="recip")
                    nc.vector.reciprocal_approx_fast(recip, po[:, N:2 * N])
                    nc.vector.tensor_mul(ao_sb[:, g, :], in0=po[:, 0:N],
                                         in1=recip)

                # ---- projection: y^T [c_out, t] ----
                for cb in range(4):
                    ps = ps_mm.tile([128, 512], F32, name="ps_g", tag="ps_g")
                    for ci in range(4):
                        nc.tensor.matmul(
                            ps[:, :N],
                            lhsT=wp_sb[:, ci, cb * 128:(cb + 1) * 128],
                            rhs=ao_sb[:, ci, :],
                            start=(ci == 0),
                            stop=(ci == 3),
                        )
                    y_sb = yp.tile([128, N], F32, name="y_sb")
                    if o["y_copy"] == "act":
                        nc.scalar.activation(y_sb, ps[:, :N], AF.Identity,
                                             bias=pb_sb[:, cb:cb + 1])
                    else:
                        nc.vector.tensor_scalar_add(
                            y_sb, ps[:, :N], pb_sb[:, cb:cb + 1])
                    nc.sync.dma_start(out=out[cb, :, col], in_=y_sb)
    nc.finalize()
    return nc


def _prep_consts(qkv_w, qkv_b, proj_w, proj_b, rpb_table):
    w = np.array(qkv_w, dtype=np.float32)
    bqkv = np.array(qkv_b, dtype=np.float32)
    w[:C] *= SCALE
    bqkv = bqkv.copy()
    bqkv[:C] *= SCALE

    wt = w.T  # [512, 1536] = [c_in, c_out]
    wqk = np.ascontiguousarray(
        wt[:, :1024].reshape(4, 128, 1024).transpose(1, 0, 2)).astype(NPBF16)
    wv = np.ascontiguousarray(
        wt[:, 1024:].reshape(4, 128, 512).transpose(1, 0, 2)).astype(NPBF16)
    wpm = np.ascontiguousarray(
        proj_w.T.reshape(4, 128, 512).transpose(1, 0, 2)).astype(NPBF16)

    bias_full = rpb_table[_REL_IDX]          # [N, N, H]  (n, m, h)
    bias_hmn = bias_full.transpose(2, 1, 0)  # [H, m, n]
    if OPTS["bias_mode"] == "dve":
        bias_hmn = np.exp(bias_hmn)
    biastp_f32 = np.ascontiguousarray(
        bias_hmn.reshape(H, 2, 128, N).transpose(2, 0, 1, 3).reshape(128, H, 512)
    ).astype(np.float32)
    if OPTS["bias_dr"] and OPTS["bias_mode"] == "pe":
        fp8 = ml_dtypes.float8_e4m3
        a = biastp_f32.astype(fp8)
        bres = (biastp_f32 - a.astype(np.float32)).astype(fp8)
        biastp = np.stack([a, bres], axis=2)  # [128, H, 2, 512] fp8
        idd = np.zeros((128, 2, 128), np.float32)
        idd[np.arange(128), :, np.arange(128)] = 1.0
        extra = {"identdr": idd.astype(fp8)}
    else:
        biastp = biastp_f32.astype(NPBF16)
        extra = {}

    modt = _MOD.T  # symmetric anyway
    modtp = np.ascontiguousarray(
        modt.reshape(2, 128, N).transpose(1, 0, 2).reshape(128, 512)).astype(NPBF16)

    bvb = np.broadcast_to(bqkv[1024:], (128, 512)).astype(NPBF16)
    qkbm = np.ascontiguousarray(
        bqkv[:1024].reshape(8, 128).T).astype(np.float32)  # [128, 8]
    pbm = np.ascontiguousarray(
        np.array(proj_b, dtype=np.float32).reshape(4, 128).T)  # [128, 4]

    return dict(wqk=wqk, wv=wv, wp=wpm, biastp=biastp, modtp=modtp,
                bvb=bvb, qkb=qkbm, pb=pbm, **extra)


def kernel(x, qkv_w, qkv_b, proj_w, proj_b, rpb_table, _trace=False):
    x = np.asarray(x, dtype=np.float32)
    consts = _prep_consts(
        np.asarray(qkv_w, np.float32), np.asarray(qkv_b, np.float32),
        np.asarray(proj_w, np.float32), np.asarray(proj_b, np.float32),
        np.asarray(rpb_table, np.float32))

    if "nc" not in _CACHE:
        _CACHE["nc"] = _build()
    nc = _CACHE["nc"]

    in_maps = []
    for i in range(NCORES):
        xs = x[i * B_LOC:(i + 1) * B_LOC].reshape(T_LOC, C)
        xtp = np.ascontiguousarray(
            xs.T.reshape(4, 128, T_LOC).transpose(1, 0, 2)).astype(NPBF16)
        in_maps.append({"xt": xtp, **consts})

    res = run_bass_kernel_spmd(nc, in_maps, core_ids=list(range(NCORES)),
                               trace=_trace)
    ys = []
    for i in range(NCORES):
        yt = np.asarray(res.results[i]["out"], np.float32)  # [4, 128, T_LOC]
        ys.append(yt.reshape(C, T_LOC).T.reshape(B_LOC, N, C))
    out = np.concatenate(ys, axis=0)
    if _trace:
        return out, res
    return out



# revision 9
# speedup vs baseline: 1.4405x; 1.4405x over previous
"""Trainium2 Bass kernel for nn_AttentionBlock (Swin-style window attention,
16x16 windows, 16 heads, head_dim 32, cosine-distance post-softmax modulation).

v2 design (data-parallel over 8 cores, 16 windows each), engine-balanced:

  PE  : qkv (pair-batched), v (no bias matmul), QK as 4x32-row-band
        concurrent bursts, PV + denominator(ones) as 4x32-col-band
        concurrent bursts, proj (pair-batched).  Relative-position bias
        for the first `xg` head-groups is injected with bf16 identity
        matmuls (exact, shares the identity stationary); remaining groups
        apply exp(bias) post-exp on DVE/GPSIMD.
  ACT : softmax exp (2 heads per ACTIVATE), qkv PSUM->SBUF copybacks
        (Identity activation with per-partition bias).
  DVE : t3 = p0*mod (and t2 = p0*exp(bias) for non-PE-bias heads),
        v copyback (+bias), softmax reciprocal + normalize, y copyback.
  GPSIMD: a tunable share of the t3/t2 elementwise multiplies.

The per-window work is software-pipelined: iteration w issues QK(w),
PV(w-1), and slices of qkv/v/proj for neighboring windows inside four
per-group "slots" so the PE never waits on the exp->mul chain and PSUM
banks (8 total: 4 QK + 2 matmul + 2 out) rotate safely.
"""

import math
import sys

import numpy as np

for _p in ("/opt/trn_rl_repo",):
    if _p not in sys.path:
        sys.path.insert(0, _p)

import ml_dtypes  # noqa: E402

import concourse.bass as bass  # noqa: E402
import concourse.mybir as mybir  # noqa: E402
from concourse import bacc, tile  # noqa: E402
from concourse.bass_utils import run_bass_kernel_spmd  # noqa: E402
from concourse.masks import make_identity  # noqa: E402
from concourse.tile_rust import add_dep_helper  # noqa: E402

BF16 = mybir.dt.bfloat16
F32 = mybir.dt.float32
NPBF16 = ml_dtypes.bfloat16

R = 16          # window side
N = R * R       # tokens per window = 256
H = 16          # heads
D = 32          # head dim
C = H * D       # 512
B_GLOB = 128    # windows total
NCORES = 8
B_LOC = B_GLOB // NCORES   # 16 windows per core
T_LOC = B_LOC * N          # 4096 tokens per core
SCALE = D ** -0.5

OPTS = {
    "xg": 2,        # head-groups (of 4) with bias via PE identity matmul
    "gp_t3": 2,     # of the 8 per-window t3 muls, how many go to GPSIMD
    "gp_t2": 1,     # of the t2 muls (4 per window at xg=2), how many on GPSIMD
}


def _rel_pos_index(r):
    coords = np.stack(np.meshgrid(np.arange(r), np.arange(r), indexing="ij"))
    cf = coords.reshape(2, -1)
    rel = cf[:, :, None] - cf[:, None, :]
    rel = rel.transpose(1, 2, 0).astype(np.int64)
    rel[:, :, 0] += r - 1
    rel[:, :, 1] += r - 1
    rel[:, :, 0] *= 2 * r - 1
    return rel.sum(-1)  # [N, N]


def _modulation(n, k):
    idx = np.arange(n * n)
    rr, cc = idx // n, idx % n
    d = np.sqrt((rr[:, None] - rr[None, :]) ** 2 + (cc[:, None] - cc[None, :]) ** 2)
    t = 4 * (n - 1) * math.sqrt(2)
    f = 2 * math.pi / t
    m = np.exp(np.cos(f * d)) / 2
    if k % n == 0:
        k = k - 1
    bound = m[0, k]
    m = np.where(m < bound, 0.0, m)
    return m.astype(np.float32)  # [N, N]


_REL_IDX = _rel_pos_index(R)
_MOD = _modulation(R, 3 * R)

_CACHE = {}


def _fold128(a):
    """[m(256), X] -> [128, 2, X] m-fold (m = c*128 + p) packed per partition."""
    x = a.reshape(2, 128, a.shape[-1]).transpose(1, 0, 2)
    return np.ascontiguousarray(x)


def _build():
    o = OPTS
    xg = o["xg"]
    n_pe_h = 4 * xg
    n_dve_h = H - n_pe_h
    AF = mybir.ActivationFunctionType

    nc = bacc.Bacc(None, target_bir_lowering=False)

    xt = nc.declare_dram_parameter("xt", [128, 4, T_LOC], BF16, isOutput=False)
    wqk = nc.declare_dram_parameter("wqk", [128, 4, 1024], BF16, isOutput=False)
    wv = nc.declare_dram_parameter("wv", [128, 4, 512], BF16, isOutput=False)
    wp = nc.declare_dram_parameter("wp", [128, 4, 512], BF16, isOutput=False)
    qkb = nc.declare_dram_parameter("qkb", [128, 8], F32, isOutput=False)
    bvb = nc.declare_dram_parameter("bvb", [128, 512], BF16, isOutput=False)
    pb = nc.declare_dram_parameter("pb", [128, 4], F32, isOutput=False)
    if n_pe_h:
        biastp = nc.declare_dram_parameter("biastp", [128, n_pe_h, 512], BF16,
                                           isOutput=False)
    if n_dve_h:
        expbtp = nc.declare_dram_parameter("expbtp", [128, n_dve_h, 512], BF16,
                                           isOutput=False)
        modexpbtp = nc.declare_dram_parameter("modexpbtp", [128, n_dve_h, 512],
                                              BF16, isOutput=False)
    modtp2 = nc.declare_dram_parameter("modtp2", [128, 2, 512], BF16,
                                       isOutput=False)
    out = nc.declare_dram_parameter("out", [4, 128, T_LOC], F32, isOutput=True)

    with tile.TileContext(nc) as tc:
        with (
            tc.tile_pool(name="const", bufs=1) as const,
            tc.tile_pool(name="qkp", bufs=2) as qkp,
            tc.tile_pool(name="vp", bufs=4) as vp,
            tc.tile_pool(name="p0p", bufs=8) as p0p,
            tc.tile_pool(name="t3p", bufs=8) as t3p,
            tc.tile_pool(name="t2p", bufs=4) as t2p,
            tc.tile_pool(name="rcp", bufs=2) as rcp,
            tc.tile_pool(name="aop", bufs=2) as aop,
            tc.tile_pool(name="yp", bufs=3) as yp,
            tc.tile_pool(name="ps_mm", bufs=2, space="PSUM") as ps_mm,
            tc.tile_pool(name="ps_s", bufs=2, space="PSUM") as ps_s,
            tc.tile_pool(name="ps_o", bufs=2, space="PSUM") as ps_o,
        ):
            # ---- resident constants ----
            wqk_sb = const.tile([128, 4, 1024], BF16, name="wqk_sb")
            nc.sync.dma_start(out=wqk_sb[:], in_=wqk[:])
            wv_sb = const.tile([128, 4, 512], BF16, name="wv_sb")
            nc.sync.dma_start(out=wv_sb[:], in_=wv[:])
            wp_sb = const.tile([128, 4, 512], BF16, name="wp_sb")
            nc.sync.dma_start(out=wp_sb[:], in_=wp[:])
            qkb_sb = const.tile([128, 8], F32, name="qkb_sb")
            nc.sync.dma_start(out=qkb_sb[:], in_=qkb[:])
            bvb_sb = const.tile([128, 512], BF16, name="bvb_sb")
            nc.sync.dma_start(out=bvb_sb[:], in_=bvb[:])
            pb_sb = const.tile([128, 4], F32, name="pb_sb")
            nc.sync.dma_start(out=pb_sb[:], in_=pb[:])
            if n_pe_h:
                biastp_sb = const.tile([128, n_pe_h, 512], BF16,
                                       name="biastp_sb")
                nc.sync.dma_start(out=biastp_sb[:], in_=biastp[:])
            if n_dve_h:
                expbtp_sb = const.tile([128, n_dve_h, 512], BF16,
                                       name="expbtp_sb")
                nc.sync.dma_start(out=expbtp_sb[:], in_=expbtp[:])
                modexpbtp_sb = const.tile([128, n_dve_h, 512], BF16,
                                          name="modexpbtp_sb")
                nc.sync.dma_start(out=modexpbtp_sb[:], in_=modexpbtp[:])
            modtp2_sb = const.tile([128, 2, 512], BF16, name="modtp2_sb")
            nc.sync.dma_start(out=modtp2_sb[:], in_=modtp2[:])
            xt_sb = const.tile([128, 4, T_LOC], BF16, name="xt_sb")
            for _b in range(B_LOC):
                _c = slice(_b * N, (_b + 1) * N)
                nc.sync.dma_start(out=xt_sb[:, :, _c], in_=xt[:, :, _c])

            ident = const.tile([128, 128], BF16, name="ident")
            make_identity(nc, ident)
            ones32 = const.tile([128, 32], BF16, name="ones32")
            nc.gpsimd.memset(ones32, 1.0)
            zeros128 = const.tile([128, 128], BF16, name="zeros128")
            nc.gpsimd.memset(zeros128, 0.0)

            # per-window saved tiles (rotating python-side handles)
            v_tiles = {}
            p0_tiles = {}   # (w, half) -> tile [128,1024] (2 heads)
            t2_tiles = {}
            t3_tiles = {}
            ao_tiles = {}   # pair P -> tile [128, 4, 512]

            def do_qkv_half(P, half):
                """qkv for window-pair P, cb blocks half*4 .. half*4+3."""
                col2 = slice(P * 512, (P + 1) * 512)
                qk_sb = qkv_bufs[P % 2]
                for cb in range(half * 4, half * 4 + 4):
                    ps = ps_mm.tile([128, 512], F32, name="ps_g", tag="ps_g")
                    for ci in range(4):
                        nc.tensor.matmul(
                            ps,
                            lhsT=wqk_sb[:, ci, cb * 128:(cb + 1) * 128],
                            rhs=xt_sb[:, ci, col2],
                            start=(ci == 0),
                            stop=(ci == 3),
                        )
                    nc.scalar.activation(qk_sb[:, cb, :], ps, AF.Identity,
                                         bias=qkb_sb[:, cb:cb + 1])

            def do_v(w, tb):
                if (w, 0) not in v_tiles and tb == 0:
                    v_tiles[w, 0] = vp.tile([128, 2, 512], BF16, name="v_sb")
                v_sb = v_tiles[w, 0]
                ps = ps_mm.tile([128, 512], F32, name="ps_g", tag="ps_g")
                tcol = slice(w * N + tb * 128, w * N + (tb + 1) * 128)
                for ci in range(4):
                    nc.tensor.matmul(
                        ps,
                        lhsT=xt_sb[:, ci, tcol],
                        rhs=wv_sb[:, ci, :],
                        start=(ci == 0),
                        stop=(ci == 3),
                    )
                nc.vector.tensor_add(v_sb[:, tb, :], ps, bvb_sb)

            def do_qk(w, g):
                """S^T (+bias for g<xg) for heads 4g..4g+3 of window w."""
                qk_sb = qkv_bufs[(w // 2) % 2]
                e2 = w % 2
                bias_pe = g < xg
                ss = [ps_s.tile([128, 2, 512], F32, name="ps_sT")
                      for _ in range(2)]
                if bias_pe:
                    for j in range(4):
                        h = 4 * g + j
                        nc.tensor.matmul(
                            ss[j // 2][:, j % 2, :],
                            lhsT=ident, rhs=biastp_sb[:, h, :],
                            start=True, stop=False)
                for c in range(2):
                    for j in range(4):
                        nc.tensor.matmul(
                            ss[j // 2][:, j % 2, c * N:(c + 1) * N],
                            lhsT=qk_sb[32 * j:32 * (j + 1), 4 + g,
                                       e2 * N + c * 128:e2 * N + (c + 1) * 128],
                            rhs=qk_sb[32 * j:32 * (j + 1), g,
                                      e2 * N:(e2 + 1) * N],
                            start=(not bias_pe and c == 0),
                            stop=(c == 1),
                            tile_position=(32 * j, 0),
                            skip_group_check=not bias_pe,
                        )
                for half in range(2):
                    p0 = p0p.tile([128, 2, 512], BF16, name="p0")
                    nc.scalar.activation(p0, ss[half], AF.Exp)
                    p0_tiles[w, g, half] = p0

            def do_muls(w, g):
                """t3 (and t2) for heads of (w, g); round-robin some on gpsimd."""
                bias_pe = g < xg
                for half in range(2):
                    p0 = p0_tiles[w, g, half]
                    t3 = t3p.tile([128, 2, 512], BF16, name="t3", tag="t3")
                    idx = g * 2 + half
                    eng3 = nc.gpsimd if idx < OPTS["gp_t3"] else nc.vector
                    if bias_pe:
                        eng3.tensor_mul(t3, p0, modtp2_sb)
                    else:
                        dh = 4 * (g - xg) + 2 * half  # index into dve-head tables
                        eng3.tensor_mul(t3, p0, modexpbtp_sb[:, dh:dh + 2, :])
                        t2 = t2p.tile([128, 2, 512], BF16, name="t2", tag="t2")
                        eng2 = (nc.gpsimd
                                if (g - xg) * 2 + half < OPTS["gp_t2"]
                                else nc.vector)
                        eng2.tensor_mul(t2, p0, expbtp_sb[:, dh:dh + 2, :])
                        t2_tiles[w, g, half] = t2
                    t3_tiles[w, g, half] = t3

            def do_pv(w, g):
                """PV + denominators + normalize for (w, g)."""
                bias_pe = g < xg
                v_sb = v_tiles[w, 0]
                po = ps_o.tile([128, 512], F32, name="ps_out")
                opener = nc.tensor.matmul(po[:, 0:N], lhsT=zeros128,
                                          rhs=bvb_sb[:, 0:N],
                                          start=True, stop=False,
                                          skip_group_check=True)
                for c in range(2):
                    for j in range(4):
                        h = 4 * g + j
                        t3 = t3_tiles[w, g, j // 2]
                        nc.tensor.matmul(
                            po[32 * j:32 * (j + 1), 0:N],
                            lhsT=v_sb[:, c, 32 * h:32 * (h + 1)],
                            rhs=t3[:, j % 2, c * N:(c + 1) * N],
                            start=False,
                            stop=(c == 1),
                            tile_position=(0, 32 * j),
                            skip_group_check=True,
                        )
                    for j in range(4):
                        src = (p0_tiles if bias_pe else t2_tiles)[w, g, j // 2]
                        mm = nc.tensor.matmul(
                            po[32 * j:32 * (j + 1), N:2 * N],
                            lhsT=ones32,
                            rhs=src[:, j % 2, c * N:(c + 1) * N],
                            start=False,
                            stop=(c == 1),
                            tile_position=(0, 32 * j),
                            skip_group_check=True,
                        )
                        if c == 0:
                            add_dep_helper(mm.ins, opener.ins, sync=False,
                                           reason="psum bank opener order")
                recip = rcp.tile([128, N], F32, name="recip")
                nc.vector.reciprocal_approx_fast(recip, po[:, N:2 * N])
                P = w // 2
                if (P,) not in ao_tiles:
                    ao_tiles[P,] = aop.tile([128, 4, 512], BF16, name="ao_sb")
                ao = ao_tiles[P,]
                nc.vector.tensor_mul(
                    ao[:, g, (w % 2) * N:(w % 2) * N + N], po[:, 0:N], recip)

            def do_proj_blk(P, cb):
                """projection for pair P, c_out block cb (of 4)."""
                ao = ao_tiles[P,]
                ps = ps_mm.tile([128, 512], F32, name="ps_g", tag="ps_g")
                for ci in range(4):
                    nc.tensor.matmul(
                        ps,
                        lhsT=wp_sb[:, ci, cb * 128:(cb + 1) * 128],
                        rhs=ao[:, ci, :],
                        start=(ci == 0),
                        stop=(ci == 3),
                    )
                y_sb = yp.tile([128, 512], F32, name="y_sb")
                nc.vector.tensor_scalar_add(y_sb, ps, pb_sb[:, cb:cb + 1])
                nc.sync.dma_start(out=out[cb, :, P * 512:(P + 1) * 512],
                                  in_=y_sb)

            qkv_bufs = [const.tile([128, 8, 512], BF16, name=f"qkbuf{i}")
                        for i in range(2)]

            # ---- prologue: pair 0 qkv, v(0) ----
            do_qkv_half(0, 0)
            do_qkv_half(0, 1)
            do_v(0, 0)
            do_v(0, 1)

            # ---- steady-state pipeline ----
            for w in range(B_LOC + 3):
                for g in range(4):
                    if w <= B_LOC - 1:
                        do_qk(w, g)
                        do_muls(w, g)
                    if 1 <= w <= B_LOC:
                        do_pv(w - 1, g)
                    # dense slices
                    if g == 0 and w % 2 == 0 and 0 <= w <= 2 * (B_LOC // 2) - 4:
                        do_qkv_half(w // 2 + 1, 0)
                    if g == 1 and w % 2 == 1 and w <= B_LOC - 3:
                        do_qkv_half((w + 1) // 2, 1)
                    if g == 2 and w <= B_LOC - 2:
                        do_v(w + 1, 0)
                    if g == 3 and w <= B_LOC - 2:
                        do_v(w + 1, 1)
                    if w % 2 == 1 and w >= 3:
                        P = (w - 3) // 2
                        if g == 0:
                            do_proj_blk(P, 0)
                        elif g == 2:
                            do_proj_blk(P, 1)
                    if w % 2 == 0 and w >= 4:
                        P = (w - 4) // 2
                        if g == 1:
                            do_proj_blk(P, 2)
                        elif g == 3:
                            do_proj_blk(P, 3)
    nc.finalize()
    return nc


def _prep_consts(qkv_w, qkv_b, proj_w, proj_b, rpb_table):
    o = OPTS
    xg = o["xg"]
    n_pe_h = 4 * xg
    w = np.array(qkv_w, dtype=np.float32)
    bqkv = np.array(qkv_b, dtype=np.float32).copy()
    w[:C] *= SCALE
    bqkv[:C] *= SCALE

    wt = w.T  # [512, 1536] = [c_in, c_out]
    wqk = np.ascontiguousarray(
        wt[:, :1024].reshape(4, 128, 1024).transpose(1, 0, 2)).astype(NPBF16)
    wv = np.ascontiguousarray(
        wt[:, 1024:].reshape(4, 128, 512).transpose(1, 0, 2)).astype(NPBF16)
    wpm = np.ascontiguousarray(
        proj_w.T.reshape(4, 128, 512).transpose(1, 0, 2)).astype(NPBF16)

    bias_full = np.asarray(rpb_table, np.float32)[_REL_IDX]   # [N, N, H] (n,m,h)
    bias_hmn = bias_full.transpose(2, 1, 0)                   # [H, m, n]
    consts = {}
    if n_pe_h:
        # bias_hmn[h] is [256(m), 256(n)] -> m-fold [128, 2, 256] -> [128, 512]
        bt = np.stack([_fold128(bias_hmn[h]).reshape(128, 512)
                       for h in range(n_pe_h)], axis=1)
        consts["biastp"] = np.ascontiguousarray(bt).astype(NPBF16)
    if n_pe_h < H:
        eb = np.exp(bias_hmn[n_pe_h:])                        # [nd, m, n]
        modt = _MOD.T[None]                                   # [1, m, n]
        eb_f = np.stack([_fold128(e).reshape(128, 512) for e in eb], axis=1)
        me_f = np.stack([_fold128(e).reshape(128, 512)
                         for e in (eb * modt)], axis=1)
        consts["expbtp"] = np.ascontiguousarray(eb_f).astype(NPBF16)
        consts["modexpbtp"] = np.ascontiguousarray(me_f).astype(NPBF16)

    modf = _fold128(_MOD.T).reshape(128, 512)
    consts["modtp2"] = np.ascontiguousarray(
        np.stack([modf, modf], axis=1)).astype(NPBF16)

    consts["bvb"] = np.broadcast_to(bqkv[1024:], (128, 512)).astype(NPBF16)
    consts["qkb"] = np.ascontiguousarray(
        bqkv[:1024].reshape(8, 128).T).astype(np.float32)  # [128, 8]
    consts["pb"] = np.ascontiguousarray(
        np.array(proj_b, dtype=np.float32).reshape(4, 128).T)  # [128, 4]

    return dict(wqk=wqk, wv=wv, wp=wpm, **consts)


def kernel(x, qkv_w, qkv_b, proj_w, proj_b, rpb_table, _trace=False):
    x = np.asarray(x, dtype=np.float32)
    consts = _prep_consts(
        np.asarray(qkv_w, np.float32), np.asarray(qkv_b, np.float32),
        np.asarray(proj_w, np.float32), np.asarray(proj_b, np.float32),
        np.asarray(rpb_table, np.float32))

    if "nc" not in _CACHE:
        _CACHE["nc"] = _build()
    nc = _CACHE["nc"]

    in_maps = []
    for i in range(NCORES):
        xs = x[i * B_LOC:(i + 1) * B_LOC].reshape(T_LOC, C)
        xtp = np.ascontiguousarray(
            xs.T.reshape(4, 128, T_LOC).transpose(1, 0, 2)).astype(NPBF16)
        in_maps.append({"xt": xtp, **consts})

    res = run_bass_kernel_spmd(nc, in_maps, core_ids=list(range(NCORES)),
                               trace=_trace)
    ys = []
    for i in range(NCORES):
        yt = np.asarray(res.results[i]["out"], np.float32)  # [4, 128, T_LOC]
        ys.append(yt.reshape(C, T_LOC).T.reshape(B_LOC, N, C))
    out = np.concatenate(ys, axis=0)
    if _trace:
        return out, res
    return out


# revision 13
# speedup vs baseline: 1.5944x; 1.1068x over previous
"""Trainium2 Bass kernel for nn_AttentionBlock (Swin-style window attention,
16x16 windows, 16 heads, head_dim 32, cosine-distance post-softmax modulation).

v2 design (data-parallel over 8 cores, 16 windows each), engine-balanced:

  PE  : qkv (pair-batched), v (no bias matmul), QK as 4x32-row-band
        concurrent bursts, PV + denominator(ones) as 4x32-col-band
        concurrent bursts, proj (pair-batched).  Relative-position bias
        for the first `xg` head-groups is injected with bf16 identity
        matmuls (exact, shares the identity stationary); remaining groups
        apply exp(bias) post-exp on DVE/GPSIMD.
  ACT : softmax exp (2 heads per ACTIVATE), qkv PSUM->SBUF copybacks
        (Identity activation with per-partition bias).
  DVE : t3 = p0*mod (and t2 = p0*exp(bias) for non-PE-bias heads),
        v copyback (+bias), softmax reciprocal + normalize, y copyback.
  GPSIMD: a tunable share of the t3/t2 elementwise multiplies.

The per-window work is software-pipelined: iteration w issues QK(w),
PV(w-1), and slices of qkv/v/proj for neighboring windows inside four
per-group "slots" so the PE never waits on the exp->mul chain and PSUM
banks (8 total: 4 QK + 2 matmul + 2 out) rotate safely.
"""

import math
import sys

import numpy as np

for _p in ("/opt/trn_rl_repo",):
    if _p not in sys.path:
        sys.path.insert(0, _p)

import ml_dtypes  # noqa: E402

import concourse.bass as bass  # noqa: E402
import concourse.mybir as mybir  # noqa: E402
from concourse import bacc, tile  # noqa: E402
from concourse.bass_utils import run_bass_kernel_spmd  # noqa: E402
from concourse.masks import make_identity  # noqa: E402
from concourse.tile_rust import add_dep_helper  # noqa: E402

BF16 = mybir.dt.bfloat16
F32 = mybir.dt.float32
NPBF16 = ml_dtypes.bfloat16

R = 16          # window side
N = R * R       # tokens per window = 256
H = 16          # heads
D = 32          # head dim
C = H * D       # 512
B_GLOB = 128    # windows total
NCORES = 8
B_LOC = B_GLOB // NCORES   # 16 windows per core
T_LOC = B_LOC * N          # 4096 tokens per core
SCALE = D ** -0.5

OPTS = {
    "xg": 2,        # head-groups (of 4) with bias via PE identity matmul
    "gp_t3": 3,     # of the 8 per-window t3 muls, how many go to GPSIMD
    "gp_t2": 1,     # of the t2 muls (4 per window at xg=2), how many on GPSIMD
    "opener": True, # explicit PSUM zero-opener before PV groups
}


def _rel_pos_index(r):
    coords = np.stack(np.meshgrid(np.arange(r), np.arange(r), indexing="ij"))
    cf = coords.reshape(2, -1)
    rel = cf[:, :, None] - cf[:, None, :]
    rel = rel.transpose(1, 2, 0).astype(np.int64)
    rel[:, :, 0] += r - 1
    rel[:, :, 1] += r - 1
    rel[:, :, 0] *= 2 * r - 1
    return rel.sum(-1)  # [N, N]


def _modulation(n, k):
    idx = np.arange(n * n)
    rr, cc = idx // n, idx % n
    d = np.sqrt((rr[:, None] - rr[None, :]) ** 2 + (cc[:, None] - cc[None, :]) ** 2)
    t = 4 * (n - 1) * math.sqrt(2)
    f = 2 * math.pi / t
    m = np.exp(np.cos(f * d)) / 2
    if k % n == 0:
        k = k - 1
    bound = m[0, k]
    m = np.where(m < bound, 0.0, m)
    return m.astype(np.float32)  # [N, N]


_REL_IDX = _rel_pos_index(R)
_MOD = _modulation(R, 3 * R)

_CACHE = {}


def _fold128(a):
    """[m(256), X] -> [128, 2, X] m-fold (m = c*128 + p) packed per partition."""
    x = a.reshape(2, 128, a.shape[-1]).transpose(1, 0, 2)
    return np.ascontiguousarray(x)


def _build():
    o = OPTS
    xg = o["xg"]
    n_pe_h = 4 * xg
    n_dve_h = H - n_pe_h
    AF = mybir.ActivationFunctionType

    nc = bacc.Bacc(None, target_bir_lowering=False)

    xt = nc.declare_dram_parameter("xt", [128, 4, T_LOC], BF16, isOutput=False)
    wqk = nc.declare_dram_parameter("wqk", [128, 4, 1024], BF16, isOutput=False)
    wv = nc.declare_dram_parameter("wv", [128, 4, 512], BF16, isOutput=False)
    wp = nc.declare_dram_parameter("wp", [128, 4, 512], BF16, isOutput=False)
    qkb = nc.declare_dram_parameter("qkb", [128, 8], F32, isOutput=False)
    bvb = nc.declare_dram_parameter("bvb", [128, 512], BF16, isOutput=False)
    pb = nc.declare_dram_parameter("pb", [128, 4], F32, isOutput=False)
    if n_pe_h:
        biastp = nc.declare_dram_parameter("biastp", [128, n_pe_h, 512], BF16,
                                           isOutput=False)
    if n_dve_h:
        expbtp = nc.declare_dram_parameter("expbtp", [128, n_dve_h, 512], BF16,
                                           isOutput=False)
        modexpbtp = nc.declare_dram_parameter("modexpbtp", [128, n_dve_h, 512],
                                              BF16, isOutput=False)
    modtp2 = nc.declare_dram_parameter("modtp2", [128, 2, 512], BF16,
                                       isOutput=False)
    out = nc.declare_dram_parameter("out", [4, 128, T_LOC], F32, isOutput=True)

    with tile.TileContext(nc) as tc:
        with (
            tc.tile_pool(name="const", bufs=1) as const,
            tc.tile_pool(name="qkp", bufs=2) as qkp,
            tc.tile_pool(name="vp", bufs=4) as vp,
            tc.tile_pool(name="p0p", bufs=8) as p0p,
            tc.tile_pool(name="t3p", bufs=8) as t3p,
            tc.tile_pool(name="t2p", bufs=4) as t2p,
            tc.tile_pool(name="rcp", bufs=2) as rcp,
            tc.tile_pool(name="aop", bufs=2) as aop,
            tc.tile_pool(name="yp", bufs=3) as yp,
            tc.tile_pool(name="ps_mm", bufs=2, space="PSUM") as ps_mm,
            tc.tile_pool(name="ps_s", bufs=2, space="PSUM") as ps_s,
            tc.tile_pool(name="ps_o", bufs=2, space="PSUM") as ps_o,
        ):
            # ---- resident constants ----
            wqk_sb = const.tile([128, 4, 1024], BF16, name="wqk_sb")
            nc.sync.dma_start(out=wqk_sb[:], in_=wqk[:])
            wv_sb = const.tile([128, 4, 512], BF16, name="wv_sb")
            nc.sync.dma_start(out=wv_sb[:], in_=wv[:])
            wp_sb = const.tile([128, 4, 512], BF16, name="wp_sb")
            nc.sync.dma_start(out=wp_sb[:], in_=wp[:])
            qkb_sb = const.tile([128, 8], F32, name="qkb_sb")
            nc.sync.dma_start(out=qkb_sb[:], in_=qkb[:])
            bvb_sb = const.tile([128, 512], BF16, name="bvb_sb")
            nc.sync.dma_start(out=bvb_sb[:], in_=bvb[:])
            pb_sb = const.tile([128, 4], F32, name="pb_sb")
            nc.sync.dma_start(out=pb_sb[:], in_=pb[:])
            if n_pe_h:
                biastp_sb = const.tile([128, n_pe_h, 512], BF16,
                                       name="biastp_sb")
                nc.sync.dma_start(out=biastp_sb[:], in_=biastp[:])
            if n_dve_h:
                expbtp_sb = const.tile([128, n_dve_h, 512], BF16,
                                       name="expbtp_sb")
                nc.sync.dma_start(out=expbtp_sb[:], in_=expbtp[:])
                modexpbtp_sb = const.tile([128, n_dve_h, 512], BF16,
                                          name="modexpbtp_sb")
                nc.sync.dma_start(out=modexpbtp_sb[:], in_=modexpbtp[:])
            modtp2_sb = const.tile([128, 2, 512], BF16, name="modtp2_sb")
            nc.sync.dma_start(out=modtp2_sb[:], in_=modtp2[:])
            xt_sb = const.tile([128, 4, T_LOC], BF16, name="xt_sb")
            for _b in range(B_LOC):
                _c = slice(_b * N, (_b + 1) * N)
                nc.sync.dma_start(out=xt_sb[:, :, _c], in_=xt[:, :, _c])

            ident = const.tile([128, 128], BF16, name="ident")
            make_identity(nc, ident)
            ones32 = const.tile([128, 32], BF16, name="ones32")
            nc.gpsimd.memset(ones32, 1.0)
            zeros128 = const.tile([128, 128], BF16, name="zeros128")
            nc.gpsimd.memset(zeros128, 0.0)

            # per-window saved tiles (rotating python-side handles)
            v_tiles = {}
            p0_tiles = {}   # (w, half) -> tile [128,1024] (2 heads)
            t2_tiles = {}
            t3_tiles = {}
            ao_tiles = {}   # pair P -> tile [128, 4, 512]

            def do_qkv_cb(P, cb):
                """qkv for window-pair P, single c_out block cb (of 8)."""
                col2 = slice(P * 512, (P + 1) * 512)
                qk_sb = qkv_bufs[P % 2]
                ps = ps_mm.tile([128, 512], F32, name="ps_g", tag="ps_g")
                for ci in range(4):
                    nc.tensor.matmul(
                        ps,
                        lhsT=wqk_sb[:, ci, cb * 128:(cb + 1) * 128],
                        rhs=xt_sb[:, ci, col2],
                        start=(ci == 0),
                        stop=(ci == 3),
                    )
                nc.scalar.activation(qk_sb[:, cb, :], ps, AF.Identity,
                                     bias=qkb_sb[:, cb:cb + 1])

            def do_v(w, tb):
                if (w, 0) not in v_tiles and tb == 0:
                    v_tiles[w, 0] = vp.tile([128, 2, 512], BF16, name="v_sb")
                v_sb = v_tiles[w, 0]
                ps = ps_mm.tile([128, 512], F32, name="ps_g", tag="ps_g")
                tcol = slice(w * N + tb * 128, w * N + (tb + 1) * 128)
                for ci in range(4):
                    nc.tensor.matmul(
                        ps,
                        lhsT=xt_sb[:, ci, tcol],
                        rhs=wv_sb[:, ci, :],
                        start=(ci == 0),
                        stop=(ci == 3),
                    )
                nc.vector.tensor_add(v_sb[:, tb, :], ps, bvb_sb)

            def do_qk(w, g):
                """S^T (+bias for g<xg) for heads 4g..4g+3 of window w."""
                qk_sb = qkv_bufs[(w // 2) % 2]
                e2 = w % 2
                bias_pe = g < xg
                ss = [ps_s.tile([128, 2, 512], F32, name="ps_sT")
                      for _ in range(2)]
                if bias_pe:
                    for j in range(4):
                        h = 4 * g + j
                        nc.tensor.matmul(
                            ss[j // 2][:, j % 2, :],
                            lhsT=ident, rhs=biastp_sb[:, h, :],
                            start=True, stop=False)
                for c in range(2):
                    for j in range(4):
                        nc.tensor.matmul(
                            ss[j // 2][:, j % 2, c * N:(c + 1) * N],
                            lhsT=qk_sb[32 * j:32 * (j + 1), 4 + g,
                                       e2 * N + c * 128:e2 * N + (c + 1) * 128],
                            rhs=qk_sb[32 * j:32 * (j + 1), g,
                                      e2 * N:(e2 + 1) * N],
                            start=(not bias_pe and c == 0),
                            stop=(c == 1),
                            tile_position=(32 * j, 0),
                            skip_group_check=not bias_pe,
                        )
                for half in range(2):
                    p0 = p0p.tile([128, 2, 512], BF16, name="p0")
                    nc.scalar.activation(p0, ss[half], AF.Exp)
                    p0_tiles[w, g, half] = p0

            def do_muls(w, g):
                """t3 (and t2) for heads of (w, g); round-robin some on gpsimd."""
                bias_pe = g < xg
                for half in range(2):
                    p0 = p0_tiles[w, g, half]
                    t3 = t3p.tile([128, 2, 512], BF16, name="t3", tag="t3")
                    idx = g * 2 + half
                    eng3 = nc.gpsimd if idx < OPTS["gp_t3"] else nc.vector
                    if bias_pe:
                        eng3.tensor_mul(t3, p0, modtp2_sb)
                    else:
                        dh = 4 * (g - xg) + 2 * half  # index into dve-head tables
                        eng3.tensor_mul(t3, p0, modexpbtp_sb[:, dh:dh + 2, :])
                        t2 = t2p.tile([128, 2, 512], BF16, name="t2", tag="t2")
                        eng2 = (nc.gpsimd
                                if (g - xg) * 2 + half < OPTS["gp_t2"]
                                else nc.vector)
                        eng2.tensor_mul(t2, p0, expbtp_sb[:, dh:dh + 2, :])
                        t2_tiles[w, g, half] = t2
                    t3_tiles[w, g, half] = t3

            def do_pv(w, g):
                """PV + denominators + normalize for (w, g)."""
                bias_pe = g < xg
                v_sb = v_tiles[w, 0]
                po = ps_o.tile([128, 512], F32, name="ps_out")
                use_opener = OPTS["opener"]
                opener = None
                if use_opener:
                    opener = nc.tensor.matmul(po[:, 0:N], lhsT=zeros128,
                                              rhs=bvb_sb[:, 0:N],
                                              start=True, stop=False,
                                              skip_group_check=True)
                for c in range(2):
                    for j in range(4):
                        h = 4 * g + j
                        t3 = t3_tiles[w, g, j // 2]
                        nc.tensor.matmul(
                            po[32 * j:32 * (j + 1), 0:N],
                            lhsT=v_sb[:, c, 32 * h:32 * (h + 1)],
                            rhs=t3[:, j % 2, c * N:(c + 1) * N],
                            start=(not use_opener and c == 0 and j == 0),
                            stop=(c == 1),
                            tile_position=(0, 32 * j),
                            skip_group_check=True,
                        )
                    for j in range(4):
                        src = (p0_tiles if bias_pe else t2_tiles)[w, g, j // 2]
                        mm = nc.tensor.matmul(
                            po[32 * j:32 * (j + 1), N:2 * N],
                            lhsT=ones32,
                            rhs=src[:, j % 2, c * N:(c + 1) * N],
                            start=False,
                            stop=(c == 1),
                            tile_position=(0, 32 * j),
                            skip_group_check=True,
                        )
                        if c == 0 and opener is not None:
                            add_dep_helper(mm.ins, opener.ins, sync=False,
                                           reason="psum bank opener order")
                recip = rcp.tile([128, N], F32, name="recip")
                nc.vector.reciprocal_approx_fast(recip, po[:, N:2 * N])
                P = w // 2
                if (P,) not in ao_tiles:
                    ao_tiles[P,] = aop.tile([128, 4, 512], BF16, name="ao_sb")
                ao = ao_tiles[P,]
                nc.vector.tensor_mul(
                    ao[:, g, (w % 2) * N:(w % 2) * N + N], po[:, 0:N], recip)

            def do_proj_blk(P, cb):
                """projection for pair P, c_out block cb (of 4)."""
                ao = ao_tiles[P,]
                ps = ps_mm.tile([128, 512], F32, name="ps_g", tag="ps_g")
                for ci in range(4):
                    nc.tensor.matmul(
                        ps,
                        lhsT=wp_sb[:, ci, cb * 128:(cb + 1) * 128],
                        rhs=ao[:, ci, :],
                        start=(ci == 0),
                        stop=(ci == 3),
                    )
                y_sb = yp.tile([128, 512], F32, name="y_sb")
                nc.vector.tensor_scalar_add(y_sb, ps, pb_sb[:, cb:cb + 1])
                nc.sync.dma_start(out=out[cb, :, P * 512:(P + 1) * 512],
                                  in_=y_sb)

            qkv_bufs = [const.tile([128, 8, 512], BF16, name=f"qkbuf{i}")
                        for i in range(2)]

            # ---- prologue: pair 0 qkv, v(0) ----
            for cb in range(8):
                do_qkv_cb(0, cb)
            do_v(0, 0)
            do_v(0, 1)

            # ---- steady-state pipeline: 4 slots per window iteration ----
            for w in range(B_LOC + 3):
                for g in range(4):
                    if w <= B_LOC - 1:
                        do_qk(w, g)
                        do_muls(w, g)
                    if 1 <= w <= B_LOC:
                        do_pv(w - 1, g)
                    # dense slices, ~2 x 512-col groups per slot
                    if w % 2 == 0 and w <= B_LOC - 4:
                        do_qkv_cb(w // 2 + 1, g)
                    if w % 2 == 1 and w <= B_LOC - 3:
                        do_qkv_cb((w + 1) // 2, 4 + g)
                    if g == 1 and w <= B_LOC - 2:
                        do_v(w + 1, 0)
                    if g == 3 and w <= B_LOC - 2:
                        do_v(w + 1, 1)
                    if w % 2 == 1 and w >= 3:
                        P = (w - 3) // 2
                        if g == 0:
                            do_proj_blk(P, 0)
                        elif g == 2:
                            do_proj_blk(P, 1)
                    if w % 2 == 0 and w >= 4:
                        P = (w - 4) // 2
                        if g == 1:
                            do_proj_blk(P, 2)
                        elif g == 3:
                            do_proj_blk(P, 3)
    nc.finalize()
    return nc


def _prep_consts(qkv_w, qkv_b, proj_w, proj_b, rpb_table):
    o = OPTS
    xg = o["xg"]
    n_pe_h = 4 * xg
    w = np.array(qkv_w, dtype=np.float32)
    bqkv = np.array(qkv_b, dtype=np.float32).copy()
    w[:C] *= SCALE
    bqkv[:C] *= SCALE

    wt = w.T  # [512, 1536] = [c_in, c_out]
    wqk = np.ascontiguousarray(
        wt[:, :1024].reshape(4, 128, 1024).transpose(1, 0, 2)).astype(NPBF16)
    wv = np.ascontiguousarray(
        wt[:, 1024:].reshape(4, 128, 512).transpose(1, 0, 2)).astype(NPBF16)
    wpm = np.ascontiguousarray(
        proj_w.T.reshape(4, 128, 512).transpose(1, 0, 2)).astype(NPBF16)

    bias_full = np.asarray(rpb_table, np.float32)[_REL_IDX]   # [N, N, H] (n,m,h)
    bias_hmn = bias_full.transpose(2, 1, 0)                   # [H, m, n]
    consts = {}
    if n_pe_h:
        # bias_hmn[h] is [256(m), 256(n)] -> m-fold [128, 2, 256] -> [128, 512]
        bt = np.stack([_fold128(bias_hmn[h]).reshape(128, 512)
                       for h in range(n_pe_h)], axis=1)
        consts["biastp"] = np.ascontiguousarray(bt).astype(NPBF16)
    if n_pe_h < H:
        eb = np.exp(bias_hmn[n_pe_h:])                        # [nd, m, n]
        modt = _MOD.T[None]                                   # [1, m, n]
        eb_f = np.stack([_fold128(e).reshape(128, 512) for e in eb], axis=1)
        me_f = np.stack([_fold128(e).reshape(128, 512)
                         for e in (eb * modt)], axis=1)
        consts["expbtp"] = np.ascontiguousarray(eb_f).astype(NPBF16)
        consts["modexpbtp"] = np.ascontiguousarray(me_f).astype(NPBF16)

    modf = _fold128(_MOD.T).reshape(128, 512)
    consts["modtp2"] = np.ascontiguousarray(
        np.stack([modf, modf], axis=1)).astype(NPBF16)

    consts["bvb"] = np.broadcast_to(bqkv[1024:], (128, 512)).astype(NPBF16)
    consts["qkb"] = np.ascontiguousarray(
        bqkv[:1024].reshape(8, 128).T).astype(np.float32)  # [128, 8]
    consts["pb"] = np.ascontiguousarray(
        np.array(proj_b, dtype=np.float32).reshape(4, 128).T)  # [128, 4]

    return dict(wqk=wqk, wv=wv, wp=wpm, **consts)


def kernel(x, qkv_w, qkv_b, proj_w, proj_b, rpb_table, _trace=False):
    x = np.asarray(x, dtype=np.float32)
    consts = _prep_consts(
        np.asarray(qkv_w, np.float32), np.asarray(qkv_b, np.float32),
        np.asarray(proj_w, np.float32), np.asarray(proj_b, np.float32),
        np.asarray(rpb_table, np.float32))

    if "nc" not in _CACHE:
        _CACHE["nc"] = _build()
    nc = _CACHE["nc"]

    in_maps = []
    for i in range(NCORES):
        xs = x[i * B_LOC:(i + 1) * B_LOC].reshape(T_LOC, C)
        xtp = np.ascontiguousarray(
            xs.T.reshape(4, 128, T_LOC).transpose(1, 0, 2)).astype(NPBF16)
        in_maps.append({"xt": xtp, **consts})

    res = run_bass_kernel_spmd(nc, in_maps, core_ids=list(range(NCORES)),
                               trace=_trace)
    ys = []
    for i in range(NCORES):
        yt = np.asarray(res.results[i]["out"], np.float32)  # [4, 128, T_LOC]
        ys.append(yt.reshape(C, T_LOC).T.reshape(B_LOC, N, C))
    out = np.concatenate(ys, axis=0)
    if _trace:
        return out, res
    return out
